# revision 15
# baseline (speedup 1.0000x reference)
"""HardTripletLoss (non-hardest branch) on 8 TRN2 NeuronCores.

Math:  loss = mean_{i!=j} relu(d_pos[i] - pdist[i,j] + margin)
  pdist[i,j] = ||x_i||^2 + ||y_j||^2 - 2 x_i.y_j ,  d_pos = diag(pdist)
  =>  per-term: relu(G[i,j] + a[i] - b[j]) with G = 2 x y^T,
      a[i] = margin + b[i] - G[i,i],  b[j] = ||y_j||^2  (xx cancels).
Diagonal (i==j) evaluates to ~relu(margin) = margin; the full unmasked sum is
computed and N*margin subtracted on the host.

Sharding: x rows split across 8 cores, y replicated.  Inputs arrive
pre-transposed/scaled from the host (bf16): xT2 = (2 x_shard)^T [128,1024],
ylT = y_shard^T [128,1024] (bit-identical to the matching yT slice),
yT = y^T [128,8192].  ~2.5 MB HBM per core, all HWDGE.

Per core, per col-pair (4 pairs of 1024-col groups), per m-tile (8):
  PSUM[128,2048] = xT2_m^T @ [yT_n | yT_n1]  chained with  -ones128^T @ sq
  (sq_n = square(yT_n) bf16, prefetched on ACT/DVE alternating)
  so PSUM = G - b over two col groups.  One epilogue op per m-supertile,
  alternating engines (same m => uniform per-partition a):
    DVE: sum_j max(PSUM + a_m, 0)   (STT vs a zeros tile, accum)
    ACT: sum_j relu(PSUM + a_m)     (activation w/ bias, accum)
a-path: sqc = square(ylT); prod = xT2*ylT elementwise; PE computes
bmz = ones@sqc - ones@prod (= b - z2) in one PSUM accumulation;
a16[1,1024] = Identity(bmz[0,:] + margin) bf16 in one ACT op;
PE-transpose (lhsT=a16 chunk, rhs=[1,1] ones) -> acol [128,8] f32.
The fold -ones@sq is the exact negation of +ones@sq (same PE summation
order), keeping a and the epilogue b consistent for the diagonal.
Host: loss = (sum(res) - N*margin) / N^2 in f64.
"""

import sys

if "/opt/trn_rl_repo" not in sys.path:
    sys.path.insert(0, "/opt/trn_rl_repo")

import numpy as np

N, D = 8192, 128
NCORES = 8
SH = N // NCORES          # 1024 x-rows per core
MT = SH // 128            # 8 m-tiles
NG = N // 1024            # 8 col groups of 1024
MARGIN = 0.2
# m-tile -> engine: even m = DVE (max vs 0), odd m = ACT (relu w/ bias).
# ACT tile first in each col (ACT also owns half the squares).
DVE_MS = tuple(m for m in range(MT) if m % 2 == 0)
ACT_MS = [m for m in range(MT) if m not in DVE_MS]
M_ORDER = [1, 0, 3, 2, 5, 4, 7, 6]

_cache = {}


def _build():
    import concourse.mybir as mybir
    from concourse import bacc
    from concourse.tile import TileContext
    from concourse.bass import ts

    f32 = mybir.dt.float32
    bf16 = mybir.dt.bfloat16
    Alu = mybir.AluOpType
    Act = mybir.ActivationFunctionType

    nc = bacc.Bacc()
    xT_in = nc.declare_dram_parameter("xT2", [128, SH], bf16, isOutput=False)
    ylT_in = nc.declare_dram_parameter("ylT", [128, SH], bf16, isOutput=False)
    yT_in = nc.declare_dram_parameter("yT", [128, N], bf16, isOutput=False)
    out_res = nc.declare_dram_parameter("res", [128, MT * (NG // 2)], f32, isOutput=True)
    NDVE = len(DVE_MS)

    def sq_engine(n):
        return "act" if n % 2 == 0 else "dve"

    with TileContext(nc) as tc:
        with (
            tc.tile_pool(name="big", bufs=1) as big,
            tc.tile_pool(name="work", bufs=3) as work,
            tc.tile_pool(name="ps", bufs=1, space="PSUM") as ps,
        ):
            yTs = [big.tile([128, 1024], bf16, name=f"yT{n}") for n in range(NG)]
            xT = big.tile([128, SH], bf16)
            ylT = big.tile([128, SH], bf16)
            sqs = [big.tile([128, 1024], bf16, name=f"sq{n}") for n in range(NG)]
            sqc = big.tile([128, SH], bf16)
            ones128 = big.tile([128, 128], bf16)
            negones = big.tile([128, 128], bf16)
            ones1 = big.tile([1, 1], bf16)
            marg = big.tile([1, 1], f32)
            zeros = big.tile([128, 2048], f32)
            prod = big.tile([128, SH], bf16)
            a16 = big.tile([1, SH], bf16)
            z2r = big.tile([1, SH], f32)
            acol = big.tile([128, MT], f32)
            res_d = big.tile([128, NDVE * (NG // 2)], f32)
            res_a = big.tile([128, (MT - NDVE) * (NG // 2)], f32)

            nc.gpsimd.memset(ones128[:], 1.0)
            nc.gpsimd.memset(negones[:], -1.0)
            nc.gpsimd.memset(ones1[:], 1.0)
            nc.gpsimd.memset(marg[:], MARGIN)
            nc.gpsimd.memset(zeros[:], 0.0)

            nc.sync.dma_start(xT[:], xT_in[:])
            nc.sync.dma_start(ylT[:], ylT_in[:])
            for n in range(NG):
                nc.sync.dma_start(yTs[n][:], yT_in[:, ts(n, 1024)])

            def do_sq(n):
                if sq_engine(n) == "act":
                    nc.scalar.activation(sqs[n][:], yTs[n][:], Act.Square)
                else:
                    nc.vector.scalar_tensor_tensor(
                        out=sqs[n][:], in0=yTs[n][:],
                        scalar=1.0, in1=yTs[n][:],
                        op0=Alu.mult, op1=Alu.mult,
                    )

            # ---- preamble: a-path first, then first squares ----
            nc.scalar.activation(sqc[:], ylT[:], Act.Square)
            nc.vector.scalar_tensor_tensor(     # prod on DVE
                out=prod[:], in0=xT[:], scalar=1.0, in1=ylT[:],
                op0=Alu.mult, op1=Alu.mult,
            )
            do_sq(0)                     # ACT
            do_sq(1)                     # DVE

            # bmz = b - z2 in one PSUM accumulation
            bmz = ps.tile([128, 2048], f32, tag="g", bufs=2)
            for h in range(2):
                nc.tensor.matmul(
                    bmz[:, ts(h, 512)], lhsT=ones128[:],
                    rhs=sqc[:, ts(h, 512)], start=True, stop=False,
                )
                nc.tensor.matmul(
                    bmz[:, ts(h, 512)], lhsT=negones[:],
                    rhs=prod[:, ts(h, 512)], start=False, stop=True,
                )
            nc.scalar.activation(
                a16[0:1, :], bmz[0:1, 0:SH], Act.Identity, bias=marg[0:1, 0:1]
            )
            # transpose a16 -> acol via PE, single copy out
            tpt = ps.tile([128, 2048], f32, tag="g", bufs=2)
            for m in range(MT):
                nc.tensor.matmul(
                    tpt[:, m : m + 1], lhsT=a16[0:1, ts(m, 128)],
                    rhs=ones1[:], start=True, stop=True,
                )
            nc.scalar.activation(acol[:, 0:MT], tpt[:, 0:MT], Act.Copy)

            # ---- main loop: col-pair supertiles, fold -b into PSUM ----
            for cp in range(NG // 2):
                cols = (2 * cp, 2 * cp + 1)
                for k, m in enumerate(M_ORDER):
                    is_dve = m in DVE_MS
                    pt = ps.tile([128, 2048], f32, tag="g", bufs=2)
                    for q, n in enumerate(cols):
                        for h in range(2):
                            sl = pt[:, q * 1024 + h * 512 : q * 1024 + (h + 1) * 512]
                            nc.tensor.matmul(
                                sl, lhsT=xT[:, ts(m, 128)],
                                rhs=yTs[n][:, ts(h, 512)],
                                start=True, stop=False,
                            )
                            nc.tensor.matmul(
                                sl, lhsT=negones[:],
                                rhs=sqs[n][:, ts(h, 512)],
                                start=False, stop=True,
                            )
                    if is_dve:
                        di = cp * NDVE + DVE_MS.index(m)
                        scr = work.tile([128, 2048], f32, tag="ep_dve", bufs=2)
                        nc.vector.scalar_tensor_tensor(
                            out=scr[:], in0=pt[:], scalar=acol[:, m : m + 1],
                            in1=zeros[:], op0=Alu.add, op1=Alu.max,
                            accum_out=res_d[:, di : di + 1],
                        )
                    else:
                        ai = cp * (MT - NDVE) + ACT_MS.index(m)
                        scr = work.tile([128, 2048], f32, tag="ep_act", bufs=2)
                        nc.scalar.activation(
                            scr[:], pt[:], Act.Relu,
                            bias=acol[:, m : m + 1],
                            accum_out=res_a[:, ai : ai + 1],
                        )
                    # prefetch the next col-pair's squares mid-pair
                    if k == 3 and 2 * cp + 2 < NG:
                        do_sq(2 * cp + 2)
                    if k == 5 and 2 * cp + 3 < NG:
                        do_sq(2 * cp + 3)

            nc.sync.dma_start(out_res[:, : NDVE * (NG // 2)], res_d[:])
            nc.sync.dma_start(out_res[:, NDVE * (NG // 2) :], res_a[:])

    return nc


def _make_in_maps(x: np.ndarray, y: np.ndarray) -> list:
    import ml_dtypes

    x = np.ascontiguousarray(x, dtype=np.float32)
    y = np.ascontiguousarray(y, dtype=np.float32)
    yb = y.astype(ml_dtypes.bfloat16)
    yT = np.ascontiguousarray(yb.T)
    in_maps = []
    for c in range(NCORES):
        sl = slice(c * SH, (c + 1) * SH)
        xT2 = np.ascontiguousarray((2.0 * x[sl]).astype(ml_dtypes.bfloat16).T)
        ylT = np.ascontiguousarray(yb[sl].T)
        in_maps.append({"xT2": xT2, "ylT": ylT, "yT": yT})
    return in_maps


def kernel(x: np.ndarray, y: np.ndarray) -> np.ndarray:
    from concourse.bass_utils import run_bass_kernel_spmd

    x = np.ascontiguousarray(x, dtype=np.float32)
    y = np.ascontiguousarray(y, dtype=np.float32)

    if "nc" not in _cache:
        nc = _build()
        if not nc.is_finalized():
            nc.finalize()
        _cache["nc"] = nc
    nc = _cache["nc"]

    out = run_bass_kernel_spmd(nc, _make_in_maps(x, y), list(range(NCORES)))
    results = out.results

    total = 0.0
    for c in range(NCORES):
        total += np.asarray(results[c]["res"], dtype=np.float64).sum()
    total -= float(N) * float(np.float32(MARGIN))
    return np.float32(total / (float(N) * float(N)))


# revision 16
# speedup vs baseline: 1.3835x; 1.3835x over previous
"""HardTripletLoss (non-hardest branch) on 8 TRN2 NeuronCores.

Math:  loss = mean_{i!=j} relu(d_pos[i] - pdist[i,j] + margin)
  pdist[i,j] = ||x_i||^2 + ||y_j||^2 - 2 x_i.y_j ,  d_pos = diag(pdist)
  =>  per-term: relu(G[i,j] + a[i] - b[j]) with G = 2 x y^T,
      a[i] = margin + b[i] - G[i,i],  b[j] = ||y_j||^2  (xx cancels).
Diagonal (i==j) evaluates to ~relu(margin) = margin; the full unmasked sum is
computed and N*margin subtracted on the host.

Sharding: x rows split across 8 cores, y replicated.  Inputs arrive
pre-transposed/scaled from the host (bf16): xT2 = (2 x_shard)^T [128,1024],
ylT = y_shard^T [128,1024] (bit-identical to the matching yT slice),
yT = y^T [128,8192].  ~2.5 MB HBM per core, all HWDGE.

Per core, per col-pair (4 pairs of 1024-col groups), per m-tile (8):
  PSUM[128,2048] = xT2_m^T @ [yT_n | yT_n1]  chained with  -ones128^T @ sq
  (sq_n = square(yT_n) bf16, prefetched on ACT/DVE alternating)
  so PSUM = G - b over two col groups.  One epilogue op per m-supertile,
  alternating engines (same m => uniform per-partition a):
    DVE: sum_j max(PSUM + a_m, 0)   (STT vs a zeros tile, accum)
    ACT: sum_j relu(PSUM + a_m)     (activation w/ bias, accum)
a-path: sqc = square(ylT); prod = xT2*ylT elementwise; PE computes
bmz = ones@sqc - ones@prod (= b - z2) in one PSUM accumulation;
a16[1,1024] = Identity(bmz[0,:] + margin) bf16 in one ACT op;
PE-transpose (lhsT=a16 chunk, rhs=[1,1] ones) -> acol [128,8] f32.
The fold -ones@sq is the exact negation of +ones@sq (same PE summation
order), keeping a and the epilogue b consistent for the diagonal.
Host: loss = (sum(res) - N*margin) / N^2 in f64.
"""

import sys

if "/opt/trn_rl_repo" not in sys.path:
    sys.path.insert(0, "/opt/trn_rl_repo")

import numpy as np

N, D = 8192, 128
NCORES = 8
SH = N // NCORES          # 1024 x-rows per core
MT = SH // 128            # 8 m-tiles
NG = N // 1024            # 8 col groups of 1024
MARGIN = 0.2
# m-tile -> engine: even m = DVE (max vs 0), odd m = ACT (relu w/ bias).
# ACT tile first in each col (ACT also owns half the squares).
DVE_MS = tuple(m for m in range(MT) if m % 2 == 0)
ACT_MS = [m for m in range(MT) if m not in DVE_MS]
M_ORDER = [1, 0, 3, 2, 5, 4, 7, 6]

_cache = {}


def _build():
    import concourse.mybir as mybir
    from concourse import bacc
    from concourse.tile import TileContext
    from concourse.bass import ts

    f32 = mybir.dt.float32
    bf16 = mybir.dt.bfloat16
    Alu = mybir.AluOpType
    Act = mybir.ActivationFunctionType

    nc = bacc.Bacc()
    xT_in = nc.declare_dram_parameter("xT2", [128, SH], bf16, isOutput=False)
    ylT_in = nc.declare_dram_parameter("ylT", [128, SH], bf16, isOutput=False)
    yT_in = nc.declare_dram_parameter("yT", [128, N], bf16, isOutput=False)
    out_res = nc.declare_dram_parameter("res", [128, MT * NG], f32, isOutput=True)
    NDVE = len(DVE_MS)

    def sq_engine(n):
        return "act" if n % 2 == 0 else "dve"

    with TileContext(nc) as tc:
        with (
            tc.tile_pool(name="big", bufs=1) as big,
            tc.tile_pool(name="work", bufs=3) as work,
            tc.tile_pool(name="ps", bufs=1, space="PSUM") as ps,
        ):
            yTs = [big.tile([128, 1024], bf16, name=f"yT{n}") for n in range(NG)]
            xT = big.tile([128, SH], bf16)
            ylT = big.tile([128, SH], bf16)
            sqs = [big.tile([128, 1024], bf16, name=f"sq{n}") for n in range(NG)]
            sqc = big.tile([128, SH], bf16)
            ones128 = big.tile([128, 128], bf16)
            negones = big.tile([128, 128], bf16)
            ones1 = big.tile([1, 1], bf16)
            marg = big.tile([1, 1], f32)
            zeros = big.tile([128, 1024], f32)
            prod = big.tile([128, SH], bf16)
            a16 = big.tile([1, SH], bf16)
            z2r = big.tile([1, SH], f32)
            acol = big.tile([128, MT], f32)
            res_d = big.tile([128, NDVE * NG], f32)
            res_a = big.tile([128, (MT - NDVE) * NG], f32)

            nc.gpsimd.memset(ones128[:], 1.0)
            nc.gpsimd.memset(negones[:], -1.0)
            nc.gpsimd.memset(ones1[:], 1.0)
            nc.gpsimd.memset(marg[:], MARGIN)
            nc.gpsimd.memset(zeros[:], 0.0)

            nc.sync.dma_start(xT[:], xT_in[:])
            nc.sync.dma_start(ylT[:], ylT_in[:])
            for n in range(NG):
                nc.sync.dma_start(yTs[n][:], yT_in[:, ts(n, 1024)])

            def do_sq(n):
                if sq_engine(n) == "act":
                    nc.scalar.activation(sqs[n][:], yTs[n][:], Act.Square)
                else:
                    nc.vector.scalar_tensor_tensor(
                        out=sqs[n][:], in0=yTs[n][:],
                        scalar=1.0, in1=yTs[n][:],
                        op0=Alu.mult, op1=Alu.mult,
                    )

            # ---- preamble: a-path first, then first squares ----
            nc.scalar.activation(sqc[:], ylT[:], Act.Square)
            nc.vector.scalar_tensor_tensor(     # prod on DVE
                out=prod[:], in0=xT[:], scalar=1.0, in1=ylT[:],
                op0=Alu.mult, op1=Alu.mult,
            )
            do_sq(0)                     # ACT
            do_sq(1)                     # DVE

            # bmz = b - z2 in one PSUM accumulation
            bmz = ps.tile([128, 1024], f32, tag="g", bufs=4)
            for h in range(2):
                nc.tensor.matmul(
                    bmz[:, ts(h, 512)], lhsT=ones128[:],
                    rhs=sqc[:, ts(h, 512)], start=True, stop=False,
                )
                nc.tensor.matmul(
                    bmz[:, ts(h, 512)], lhsT=negones[:],
                    rhs=prod[:, ts(h, 512)], start=False, stop=True,
                )
            nc.scalar.activation(
                a16[0:1, :], bmz[0:1, 0:SH], Act.Identity, bias=marg[0:1, 0:1]
            )
            # transpose a16 -> acol via PE, single copy out
            tpt = ps.tile([128, 1024], f32, tag="g", bufs=4)
            for m in range(MT):
                nc.tensor.matmul(
                    tpt[:, m : m + 1], lhsT=a16[0:1, ts(m, 128)],
                    rhs=ones1[:], start=True, stop=True,
                )
            nc.scalar.activation(acol[:, 0:MT], tpt[:, 0:MT], Act.Copy)

            # ---- main loop: per-col m-tiles, fold -b into PSUM ----
            for n in range(NG):
                for k, m in enumerate(M_ORDER):
                    is_dve = m in DVE_MS
                    pt = ps.tile([128, 1024], f32, tag="g", bufs=4)
                    # both G halves first (one weight load), then both folds
                    for h in range(2):
                        nc.tensor.matmul(
                            pt[:, ts(h, 512)], lhsT=xT[:, ts(m, 128)],
                            rhs=yTs[n][:, ts(h, 512)],
                            start=True, stop=False,
                        )
                    for h in range(2):
                        nc.tensor.matmul(
                            pt[:, ts(h, 512)], lhsT=negones[:],
                            rhs=sqs[n][:, ts(h, 512)],
                            start=False, stop=True,
                        )
                    if is_dve:
                        di = n * NDVE + DVE_MS.index(m)
                        scr = work.tile([128, 1024], f32, tag="ep_dve")
                        nc.vector.scalar_tensor_tensor(
                            out=scr[:], in0=pt[:], scalar=acol[:, m : m + 1],
                            in1=zeros[:], op0=Alu.add, op1=Alu.max,
                            accum_out=res_d[:, di : di + 1],
                        )
                    else:
                        ai = n * (MT - NDVE) + ACT_MS.index(m)
                        scr = work.tile([128, 1024], f32, tag="ep_act")
                        nc.scalar.activation(
                            scr[:], pt[:], Act.Relu,
                            bias=acol[:, m : m + 1],
                            accum_out=res_a[:, ai : ai + 1],
                        )
                    if k == 3 and n + 2 < NG:
                        do_sq(n + 2)

            nc.sync.dma_start(out_res[:, : NDVE * NG], res_d[:])
            nc.sync.dma_start(out_res[:, NDVE * NG :], res_a[:])

    return nc


def _make_in_maps(x: np.ndarray, y: np.ndarray) -> list:
    import ml_dtypes

    x = np.ascontiguousarray(x, dtype=np.float32)
    y = np.ascontiguousarray(y, dtype=np.float32)
    yb = y.astype(ml_dtypes.bfloat16)
    yT = np.ascontiguousarray(yb.T)
    in_maps = []
    for c in range(NCORES):
        sl = slice(c * SH, (c + 1) * SH)
        xT2 = np.ascontiguousarray((2.0 * x[sl]).astype(ml_dtypes.bfloat16).T)
        ylT = np.ascontiguousarray(yb[sl].T)
        in_maps.append({"xT2": xT2, "ylT": ylT, "yT": yT})
    return in_maps


def kernel(x: np.ndarray, y: np.ndarray) -> np.ndarray:
    from concourse.bass_utils import run_bass_kernel_spmd

    x = np.ascontiguousarray(x, dtype=np.float32)
    y = np.ascontiguousarray(y, dtype=np.float32)

    if "nc" not in _cache:
        nc = _build()
        if not nc.is_finalized():
            nc.finalize()
        _cache["nc"] = nc
    nc = _cache["nc"]

    out = run_bass_kernel_spmd(nc, _make_in_maps(x, y), list(range(NCORES)))
    results = out.results

    total = 0.0
    for c in range(NCORES):
        total += np.asarray(results[c]["res"], dtype=np.float64).sum()
    total -= float(N) * float(np.float32(MARGIN))
    return np.float32(total / (float(N) * float(N)))


# revision 17
# speedup vs baseline: 1.3850x; 1.0011x over previous
"""HardTripletLoss (non-hardest branch) on 8 TRN2 NeuronCores.

Math:  loss = mean_{i!=j} relu(d_pos[i] - pdist[i,j] + margin)
  pdist[i,j] = ||x_i||^2 + ||y_j||^2 - 2 x_i.y_j ,  d_pos = diag(pdist)
  =>  per-term: relu(G[i,j] + a[i] - b[j]) with G = 2 x y^T,
      a[i] = margin + b[i] - G[i,i],  b[j] = ||y_j||^2  (xx cancels).
Diagonal (i==j) evaluates to ~relu(margin) = margin; the full unmasked sum is
computed and N*margin subtracted on the host.

Sharding: x rows split across 8 cores, y replicated.  Inputs arrive
pre-transposed/scaled from the host (bf16): xT2 = (2 x_shard)^T [128,1024],
ylT = y_shard^T [128,1024] (bit-identical to the matching yT slice),
yT = y^T [128,8192].  ~2.5 MB HBM per core, all HWDGE.

Per core, per col-pair (4 pairs of 1024-col groups), per m-tile (8):
  PSUM[128,2048] = xT2_m^T @ [yT_n | yT_n1]  chained with  -ones128^T @ sq
  (sq_n = square(yT_n) bf16, prefetched on ACT/DVE alternating)
  so PSUM = G - b over two col groups.  One epilogue op per m-supertile,
  alternating engines (same m => uniform per-partition a):
    DVE: sum_j max(PSUM + a_m, 0)   (STT vs a zeros tile, accum)
    ACT: sum_j relu(PSUM + a_m)     (activation w/ bias, accum)
a-path: sqc = square(ylT); prod = xT2*ylT elementwise; PE computes
bmz = ones@sqc - ones@prod (= b - z2) in one PSUM accumulation;
a16[1,1024] = Identity(bmz[0,:] + margin) bf16 in one ACT op;
PE-transpose (lhsT=a16 chunk, rhs=[1,1] ones) -> acol [128,8] f32.
The fold -ones@sq is the exact negation of +ones@sq (same PE summation
order), keeping a and the epilogue b consistent for the diagonal.
Host: loss = (sum(res) - N*margin) / N^2 in f64.
"""

import sys

if "/opt/trn_rl_repo" not in sys.path:
    sys.path.insert(0, "/opt/trn_rl_repo")

import numpy as np

N, D = 8192, 128
NCORES = 8
SH = N // NCORES          # 1024 x-rows per core
MT = SH // 128            # 8 m-tiles
NG = N // 1024            # 8 col groups of 1024
MARGIN = 0.2
# m-tile -> engine: even m = DVE (max vs 0), odd m = ACT (relu w/ bias).
# ACT tile first in each col (ACT also owns half the squares).
DVE_MS = tuple(m for m in range(MT) if m % 2 == 0)
ACT_MS = [m for m in range(MT) if m not in DVE_MS]
M_ORDER = [1, 0, 3, 2, 5, 4, 7, 6]

_cache = {}


def _build():
    import concourse.mybir as mybir
    from concourse import bacc
    from concourse.tile import TileContext
    from concourse.bass import ts

    f32 = mybir.dt.float32
    bf16 = mybir.dt.bfloat16
    Alu = mybir.AluOpType
    Act = mybir.ActivationFunctionType

    nc = bacc.Bacc()
    xyl_in = nc.declare_dram_parameter("xyl", [128, 2 * SH], bf16, isOutput=False)
    yT_in = nc.declare_dram_parameter("yT", [128, N], bf16, isOutput=False)
    out_res = nc.declare_dram_parameter("res", [128, MT * NG], f32, isOutput=True)
    NDVE = len(DVE_MS)

    def sq_engine(n):
        return "act" if n % 2 == 0 else "dve"

    with TileContext(nc) as tc:
        with (
            tc.tile_pool(name="big", bufs=1) as big,
            tc.tile_pool(name="work", bufs=3) as work,
            tc.tile_pool(name="ps", bufs=1, space="PSUM") as ps,
        ):
            yTs = [big.tile([128, 1024], bf16, name=f"yT{n}") for n in range(NG)]
            xyl = big.tile([128, 2 * SH], bf16)
            sqs = [big.tile([128, 1024], bf16, name=f"sq{n}") for n in range(NG)]
            sqc = big.tile([128, SH], bf16)
            ones128 = big.tile([128, 128], bf16)
            negones = big.tile([128, 128], bf16)
            ones1 = big.tile([1, 1], bf16)
            marg = big.tile([1, 1], f32)
            zeros = big.tile([128, 1024], f32)
            prod = big.tile([128, SH], bf16)
            a16 = big.tile([1, SH], bf16)
            z2r = big.tile([1, SH], f32)
            acol = big.tile([128, MT], f32)
            res_d = big.tile([128, NDVE * NG], f32)
            res_a = big.tile([128, (MT - NDVE) * NG], f32)

            nc.gpsimd.memset(ones128[:], 1.0)
            nc.gpsimd.memset(negones[:], -1.0)
            nc.gpsimd.memset(ones1[:], 1.0)
            nc.gpsimd.memset(marg[:], MARGIN)
            nc.gpsimd.memset(zeros[:], 0.0)

            nc.sync.dma_start(xyl[:], xyl_in[:])
            xT = xyl[:, 0:SH]
            ylT = xyl[:, SH : 2 * SH]
            for n in range(NG):
                nc.sync.dma_start(yTs[n][:], yT_in[:, ts(n, 1024)])

            def do_sq(n):
                if sq_engine(n) == "act":
                    nc.scalar.activation(sqs[n][:], yTs[n][:], Act.Square)
                else:
                    nc.vector.scalar_tensor_tensor(
                        out=sqs[n][:], in0=yTs[n][:],
                        scalar=1.0, in1=yTs[n][:],
                        op0=Alu.mult, op1=Alu.mult,
                    )

            # ---- preamble: a-path first, then first squares ----
            nc.scalar.activation(sqc[:], ylT, Act.Square)
            nc.vector.scalar_tensor_tensor(     # prod on DVE
                out=prod[:], in0=xT, scalar=1.0, in1=ylT,
                op0=Alu.mult, op1=Alu.mult,
            )
            do_sq(0)                     # ACT
            do_sq(1)                     # DVE

            # bmz = b - z2 in one PSUM accumulation
            bmz = ps.tile([128, 1024], f32, tag="apath", bufs=1)
            for h in range(2):
                nc.tensor.matmul(
                    bmz[:, ts(h, 512)], lhsT=ones128[:],
                    rhs=sqc[:, ts(h, 512)], start=True, stop=False,
                )
                nc.tensor.matmul(
                    bmz[:, ts(h, 512)], lhsT=negones[:],
                    rhs=prod[:, ts(h, 512)], start=False, stop=True,
                )
            nc.scalar.activation(
                a16[0:1, :], bmz[0:1, 0:SH], Act.Identity, bias=marg[0:1, 0:1]
            )
            # transpose a16 -> acol via PE, single copy out
            tpt = ps.tile([128, 1024], f32, tag="apath", bufs=1)
            for m in range(MT):
                nc.tensor.matmul(
                    tpt[:, m : m + 1], lhsT=a16[0:1, ts(m, 128)],
                    rhs=ones1[:], start=True, stop=True,
                )
            nc.scalar.activation(acol[:, 0:MT], tpt[:, 0:MT], Act.Copy)

            # ---- main loop: per-col m-tiles, fold -b into PSUM ----
            for n in range(NG):
                for k, m in enumerate(M_ORDER):
                    is_dve = m in DVE_MS
                    pt = ps.tile([128, 1024], f32, tag="g", bufs=3)
                    # both G halves first (one weight load), then both folds
                    for h in range(2):
                        nc.tensor.matmul(
                            pt[:, ts(h, 512)], lhsT=xT[:, ts(m, 128)],
                            rhs=yTs[n][:, ts(h, 512)],
                            start=True, stop=False,
                        )
                    for h in range(2):
                        nc.tensor.matmul(
                            pt[:, ts(h, 512)], lhsT=negones[:],
                            rhs=sqs[n][:, ts(h, 512)],
                            start=False, stop=True,
                        )
                    if is_dve:
                        di = n * NDVE + DVE_MS.index(m)
                        scr = work.tile([128, 1024], f32, tag="ep_dve")
                        nc.vector.scalar_tensor_tensor(
                            out=scr[:], in0=pt[:], scalar=acol[:, m : m + 1],
                            in1=zeros[:], op0=Alu.add, op1=Alu.max,
                            accum_out=res_d[:, di : di + 1],
                        )
                    else:
                        ai = n * (MT - NDVE) + ACT_MS.index(m)
                        scr = work.tile([128, 1024], f32, tag="ep_act")
                        nc.scalar.activation(
                            scr[:], pt[:], Act.Relu,
                            bias=acol[:, m : m + 1],
                            accum_out=res_a[:, ai : ai + 1],
                        )
                    if k == 3 and n + 2 < NG:
                        do_sq(n + 2)

            nc.sync.dma_start(out_res[:, : NDVE * NG], res_d[:])
            nc.sync.dma_start(out_res[:, NDVE * NG :], res_a[:])

    return nc


def _make_in_maps(x: np.ndarray, y: np.ndarray) -> list:
    import ml_dtypes

    x = np.ascontiguousarray(x, dtype=np.float32)
    y = np.ascontiguousarray(y, dtype=np.float32)
    yb = y.astype(ml_dtypes.bfloat16)
    yT = np.ascontiguousarray(yb.T)
    in_maps = []
    for c in range(NCORES):
        sl = slice(c * SH, (c + 1) * SH)
        xT2 = (2.0 * x[sl]).astype(ml_dtypes.bfloat16).T
        ylT = yb[sl].T
        xyl = np.ascontiguousarray(np.concatenate([xT2, ylT], axis=1))
        in_maps.append({"xyl": xyl, "yT": yT})
    return in_maps


def kernel(x: np.ndarray, y: np.ndarray) -> np.ndarray:
    from concourse.bass_utils import run_bass_kernel_spmd

    x = np.ascontiguousarray(x, dtype=np.float32)
    y = np.ascontiguousarray(y, dtype=np.float32)

    if "nc" not in _cache:
        nc = _build()
        if not nc.is_finalized():
            nc.finalize()
        _cache["nc"] = nc
    nc = _cache["nc"]

    out = run_bass_kernel_spmd(nc, _make_in_maps(x, y), list(range(NCORES)))
    results = out.results

    total = 0.0
    for c in range(NCORES):
        total += np.asarray(results[c]["res"], dtype=np.float64).sum()
    total -= float(N) * float(np.float32(MARGIN))
    return np.float32(total / (float(N) * float(N)))


# revision 18
# speedup vs baseline: 1.3976x; 1.0091x over previous
"""HardTripletLoss (non-hardest branch) on 8 TRN2 NeuronCores.

Math:  loss = mean_{i!=j} relu(d_pos[i] - pdist[i,j] + margin)
  pdist[i,j] = ||x_i||^2 + ||y_j||^2 - 2 x_i.y_j ,  d_pos = diag(pdist)
  =>  per-term: relu(G[i,j] + a[i] - b[j]) with G = 2 x y^T,
      a[i] = margin + b[i] - G[i,i],  b[j] = ||y_j||^2  (xx cancels).
Diagonal (i==j) evaluates to ~relu(margin) = margin; the full unmasked sum is
computed and N*margin subtracted on the host.

Sharding: x rows split across 8 cores, y replicated.  Inputs arrive
pre-transposed/scaled from the host (bf16): xT2 = (2 x_shard)^T [128,1024],
ylT = y_shard^T [128,1024] (bit-identical to the matching yT slice),
yT = y^T [128,8192].  ~2.5 MB HBM per core, all HWDGE.

Per core, per col-pair (4 pairs of 1024-col groups), per m-tile (8):
  PSUM[128,2048] = xT2_m^T @ [yT_n | yT_n1]  chained with  -ones128^T @ sq
  (sq_n = square(yT_n) bf16, prefetched on ACT/DVE alternating)
  so PSUM = G - b over two col groups.  One epilogue op per m-supertile,
  alternating engines (same m => uniform per-partition a):
    DVE: sum_j max(PSUM + a_m, 0)   (STT vs a zeros tile, accum)
    ACT: sum_j relu(PSUM + a_m)     (activation w/ bias, accum)
a-path: sqc = square(ylT); prod = xT2*ylT elementwise; PE computes
bmz = ones@sqc - ones@prod (= b - z2) in one PSUM accumulation;
a16[1,1024] = Identity(bmz[0,:] + margin) bf16 in one ACT op;
PE-transpose (lhsT=a16 chunk, rhs=[1,1] ones) -> acol [128,8] f32.
The fold -ones@sq is the exact negation of +ones@sq (same PE summation
order), keeping a and the epilogue b consistent for the diagonal.
Host: loss = (sum(res) - N*margin) / N^2 in f64.
"""

import sys

if "/opt/trn_rl_repo" not in sys.path:
    sys.path.insert(0, "/opt/trn_rl_repo")

import numpy as np

N, D = 8192, 128
NCORES = 8
SH = N // NCORES          # 1024 x-rows per core
MT = SH // 128            # 8 m-tiles
NG = N // 1024            # 8 col groups of 1024
MARGIN = 0.2
# m-tile -> engine: even m = DVE (max vs 0), odd m = ACT (relu w/ bias).
# ACT tile first in each col (ACT also owns half the squares).
DVE_MS = tuple(m for m in range(MT) if m % 2 == 0)
ACT_MS = [m for m in range(MT) if m not in DVE_MS]
M_ORDER = [1, 0, 3, 2, 5, 4, 7, 6]

_cache = {}


def _build():
    import concourse.mybir as mybir
    from concourse import bacc
    from concourse.tile import TileContext
    from concourse.bass import ts

    f32 = mybir.dt.float32
    bf16 = mybir.dt.bfloat16
    Alu = mybir.AluOpType
    Act = mybir.ActivationFunctionType

    nc = bacc.Bacc()
    xyl_in = nc.declare_dram_parameter("xyl", [128, 2 * SH], bf16, isOutput=False)
    yT_in = nc.declare_dram_parameter("yT", [128, N], bf16, isOutput=False)
    out_res = nc.declare_dram_parameter("res", [128, MT * NG], f32, isOutput=True)
    NDVE = len(DVE_MS)

    def sq_engine(n):
        return "act" if n % 2 == 0 else "dve"

    with TileContext(nc) as tc:
        with (
            tc.tile_pool(name="big", bufs=1) as big,
            tc.tile_pool(name="work", bufs=3) as work,
            tc.tile_pool(name="ps", bufs=1, space="PSUM") as ps,
        ):
            yTs = [big.tile([128, 1024], bf16, name=f"yT{n}") for n in range(NG)]
            xyl = big.tile([128, 2 * SH], bf16)
            sqs = [big.tile([128, 1024], bf16, name=f"sq{n}") for n in range(NG)]
            sqc = big.tile([128, SH], bf16)
            ones128 = big.tile([128, 128], bf16)
            negones = big.tile([128, 128], bf16)
            ones1 = big.tile([1, 1], bf16)
            marg = big.tile([1, 1], f32)
            zeros = big.tile([128, 1024], f32)
            prod = big.tile([128, SH], bf16)
            a16 = big.tile([1, SH], bf16)
            z2r = big.tile([1, SH], f32)
            acol = big.tile([128, MT], f32)
            res_d = big.tile([128, NDVE * NG], f32)
            res_a = big.tile([128, (MT - NDVE) * NG], f32)

            nc.gpsimd.memset(ones128[:], 1.0)
            nc.gpsimd.memset(negones[:], -1.0)
            nc.gpsimd.memset(ones1[:], 1.0)
            nc.gpsimd.memset(marg[:], MARGIN)
            nc.gpsimd.memset(zeros[:], 0.0)

            nc.sync.dma_start(xyl[:], xyl_in[:])
            xT = xyl[:, 0:SH]
            ylT = xyl[:, SH : 2 * SH]
            for n in range(NG):
                nc.sync.dma_start(yTs[n][:], yT_in[:, ts(n, 1024)])

            def do_sq(n):
                if sq_engine(n) == "act":
                    nc.scalar.activation(sqs[n][:], yTs[n][:], Act.Square)
                else:
                    nc.vector.scalar_tensor_tensor(
                        out=sqs[n][:], in0=yTs[n][:],
                        scalar=1.0, in1=yTs[n][:],
                        op0=Alu.mult, op1=Alu.mult,
                    )

            # ---- preamble: a-path first, then first squares ----
            nc.scalar.activation(sqc[:], ylT, Act.Square)
            nc.vector.scalar_tensor_tensor(     # prod on DVE
                out=prod[:], in0=xT, scalar=1.0, in1=ylT,
                op0=Alu.mult, op1=Alu.mult,
            )
            do_sq(0)                     # ACT
            do_sq(1)                     # DVE

            # bmz = b - z2 in one PSUM accumulation
            bmz = ps.tile([128, 1024], f32, tag="g", bufs=4)
            for h in range(2):
                nc.tensor.matmul(
                    bmz[:, ts(h, 512)], lhsT=ones128[:],
                    rhs=sqc[:, ts(h, 512)], start=True, stop=False,
                )
                nc.tensor.matmul(
                    bmz[:, ts(h, 512)], lhsT=negones[:],
                    rhs=prod[:, ts(h, 512)], start=False, stop=True,
                )
            nc.scalar.activation(
                a16[0:1, :], bmz[0:1, 0:SH], Act.Identity, bias=marg[0:1, 0:1]
            )
            # transpose a16 -> acol via PE, single copy out
            tpt = ps.tile([128, 1024], f32, tag="g", bufs=4)
            for m in range(MT):
                nc.tensor.matmul(
                    tpt[:, m : m + 1], lhsT=a16[0:1, ts(m, 128)],
                    rhs=ones1[:], start=True, stop=True,
                )
            nc.scalar.activation(acol[:, 0:MT], tpt[:, 0:MT], Act.Copy)

            # ---- main loop: per-col m-tiles, fold -b into PSUM ----
            for n in range(NG):
                for k, m in enumerate(M_ORDER):
                    is_dve = m in DVE_MS
                    pt = ps.tile([128, 1024], f32, tag="g", bufs=4)
                    # both G halves first (one weight load), then both folds
                    for h in range(2):
                        nc.tensor.matmul(
                            pt[:, ts(h, 512)], lhsT=xT[:, ts(m, 128)],
                            rhs=yTs[n][:, ts(h, 512)],
                            start=True, stop=False,
                        )
                    for h in range(2):
                        nc.tensor.matmul(
                            pt[:, ts(h, 512)], lhsT=negones[:],
                            rhs=sqs[n][:, ts(h, 512)],
                            start=False, stop=True,
                        )
                    if is_dve:
                        di = n * NDVE + DVE_MS.index(m)
                        scr = work.tile([128, 1024], f32, tag="ep_dve")
                        nc.vector.scalar_tensor_tensor(
                            out=scr[:], in0=pt[:], scalar=acol[:, m : m + 1],
                            in1=zeros[:], op0=Alu.add, op1=Alu.max,
                            accum_out=res_d[:, di : di + 1],
                        )
                    else:
                        ai = n * (MT - NDVE) + ACT_MS.index(m)
                        scr = work.tile([128, 1024], f32, tag="ep_act")
                        nc.scalar.activation(
                            scr[:], pt[:], Act.Relu,
                            bias=acol[:, m : m + 1],
                            accum_out=res_a[:, ai : ai + 1],
                        )
                    if k == 6 and n + 2 < NG:
                        do_sq(n + 2)

            nc.sync.dma_start(out_res[:, : NDVE * NG], res_d[:])
            nc.sync.dma_start(out_res[:, NDVE * NG :], res_a[:])

    return nc


def _make_in_maps(x: np.ndarray, y: np.ndarray) -> list:
    import ml_dtypes

    x = np.ascontiguousarray(x, dtype=np.float32)
    y = np.ascontiguousarray(y, dtype=np.float32)
    yb = y.astype(ml_dtypes.bfloat16)
    yT = np.ascontiguousarray(yb.T)
    in_maps = []
    for c in range(NCORES):
        sl = slice(c * SH, (c + 1) * SH)
        xT2 = (2.0 * x[sl]).astype(ml_dtypes.bfloat16).T
        ylT = yb[sl].T
        xyl = np.ascontiguousarray(np.concatenate([xT2, ylT], axis=1))
        in_maps.append({"xyl": xyl, "yT": yT})
    return in_maps


def kernel(x: np.ndarray, y: np.ndarray) -> np.ndarray:
    from concourse.bass_utils import run_bass_kernel_spmd

    x = np.ascontiguousarray(x, dtype=np.float32)
    y = np.ascontiguousarray(y, dtype=np.float32)

    if "nc" not in _cache:
        nc = _build()
        if not nc.is_finalized():
            nc.finalize()
        _cache["nc"] = nc
    nc = _cache["nc"]

    out = run_bass_kernel_spmd(nc, _make_in_maps(x, y), list(range(NCORES)))
    results = out.results

    total = 0.0
    for c in range(NCORES):
        total += np.asarray(results[c]["res"], dtype=np.float64).sum()
    total -= float(N) * float(np.float32(MARGIN))
    return np.float32(total / (float(N) * float(N)))


# revision 20
# speedup vs baseline: 1.3978x; 1.0001x over previous
"""HardTripletLoss (non-hardest branch) on 8 TRN2 NeuronCores.

Math:  loss = mean_{i!=j} relu(d_pos[i] - pdist[i,j] + margin)
  pdist[i,j] = ||x_i||^2 + ||y_j||^2 - 2 x_i.y_j ,  d_pos = diag(pdist)
  =>  per-term: relu(G[i,j] + a[i] - b[j]) with G = 2 x y^T,
      a[i] = margin + b[i] - G[i,i],  b[j] = ||y_j||^2  (xx cancels).
Diagonal (i==j) evaluates to ~relu(margin) = margin; the full unmasked sum is
computed and N*margin subtracted on the host.

Sharding: x rows split across 8 cores, y replicated.  Inputs arrive
pre-transposed/scaled from the host (bf16): xT2 = (2 x_shard)^T [128,1024],
ylT = y_shard^T [128,1024] (bit-identical to the matching yT slice),
yT = y^T [128,8192].  ~2.5 MB HBM per core, all HWDGE.

Per core, per col-pair (4 pairs of 1024-col groups), per m-tile (8):
  PSUM[128,2048] = xT2_m^T @ [yT_n | yT_n1]  chained with  -ones128^T @ sq
  (sq_n = square(yT_n) bf16, prefetched on ACT/DVE alternating)
  so PSUM = G - b over two col groups.  One epilogue op per m-supertile,
  alternating engines (same m => uniform per-partition a):
    DVE: sum_j max(PSUM + a_m, 0)   (STT vs a zeros tile, accum)
    ACT: sum_j relu(PSUM + a_m)     (activation w/ bias, accum)
a-path: sqc = square(ylT); prod = xT2*ylT elementwise; PE computes
bmz = ones@sqc - ones@prod (= b - z2) in one PSUM accumulation;
a16[1,1024] = Identity(bmz[0,:] + margin) bf16 in one ACT op;
PE-transpose (lhsT=a16 chunk, rhs=[1,1] ones) -> acol [128,8] f32.
The fold -ones@sq is the exact negation of +ones@sq (same PE summation
order), keeping a and the epilogue b consistent for the diagonal.
Host: loss = (sum(res) - N*margin) / N^2 in f64.
"""

import sys

if "/opt/trn_rl_repo" not in sys.path:
    sys.path.insert(0, "/opt/trn_rl_repo")

import numpy as np

N, D = 8192, 128
NCORES = 8
SH = N // NCORES          # 1024 x-rows per core
MT = SH // 128            # 8 m-tiles
NG = N // 1024            # 8 col groups of 1024
MARGIN = 0.2
# m-tile -> engine: even m = DVE (max vs 0), odd m = ACT (relu w/ bias).
# ACT tile first in each col (ACT also owns half the squares).
DVE_MS = tuple(m for m in range(MT) if m % 2 == 0)
ACT_MS = [m for m in range(MT) if m not in DVE_MS]
M_ORDER = [1, 0, 3, 2, 5, 4, 7, 6]

_cache = {}


def _build():
    import concourse.mybir as mybir
    from concourse import bacc
    from concourse.tile import TileContext
    from concourse.bass import ts

    f32 = mybir.dt.float32
    bf16 = mybir.dt.bfloat16
    Alu = mybir.AluOpType
    Act = mybir.ActivationFunctionType

    nc = bacc.Bacc()
    xyl_in = nc.declare_dram_parameter("xyl", [128, 2 * SH], bf16, isOutput=False)
    yT_in = nc.declare_dram_parameter("yT", [128, N], bf16, isOutput=False)
    out_res = nc.declare_dram_parameter("res", [128, MT * NG], f32, isOutput=True)
    NDVE = len(DVE_MS)

    def sq_engine(n):
        return "act" if n % 2 == 0 else "dve"

    with TileContext(nc) as tc:
        with (
            tc.tile_pool(name="big", bufs=1) as big,
            tc.tile_pool(name="work", bufs=3) as work,
            tc.tile_pool(name="ps", bufs=1, space="PSUM") as ps,
        ):
            yTs = [big.tile([128, 1024], bf16, name=f"yT{n}") for n in range(NG)]
            xyl = big.tile([128, 2 * SH], bf16)
            sqs = [big.tile([128, 1024], bf16, name=f"sq{n}") for n in range(NG)]
            sqc = big.tile([128, SH], bf16)
            ones128 = big.tile([128, 128], bf16)
            negones = big.tile([128, 128], bf16)
            ones1 = big.tile([1, 1], bf16)
            marg = big.tile([1, 1], f32)
            zeros = big.tile([128, 1024], f32)
            prod = big.tile([128, SH], bf16)
            a16 = big.tile([1, SH], bf16)
            z2r = big.tile([1, SH], f32)
            acol = big.tile([128, MT], f32)
            res_d = big.tile([128, NDVE * NG], f32)
            res_a = big.tile([128, (MT - NDVE) * NG], f32)

            nc.gpsimd.memset(ones128[:], 1.0)
            nc.gpsimd.memset(negones[:], -1.0)
            nc.gpsimd.memset(ones1[:], 1.0)
            nc.gpsimd.memset(marg[:], MARGIN)
            nc.gpsimd.memset(zeros[:], 0.0)

            nc.sync.dma_start(xyl[:], xyl_in[:])
            xT = xyl[:, 0:SH]
            ylT = xyl[:, SH : 2 * SH]
            for n in range(NG):
                nc.sync.dma_start(yTs[n][:], yT_in[:, ts(n, 1024)])

            def do_sq(n):
                if sq_engine(n) == "act":
                    nc.scalar.activation(sqs[n][:], yTs[n][:], Act.Square)
                else:
                    nc.vector.scalar_tensor_tensor(
                        out=sqs[n][:], in0=yTs[n][:],
                        scalar=1.0, in1=yTs[n][:],
                        op0=Alu.mult, op1=Alu.mult,
                    )

            # ---- preamble: a-path first, then first squares ----
            nc.scalar.activation(sqc[:], ylT, Act.Square)
            nc.vector.scalar_tensor_tensor(     # prod on DVE
                out=prod[:], in0=xT, scalar=1.0, in1=ylT,
                op0=Alu.mult, op1=Alu.mult,
            )
            do_sq(0)                     # ACT
            do_sq(1)                     # DVE

            # bmz = b - z2 in one PSUM accumulation
            bmz = ps.tile([128, 1024], f32, tag="g", bufs=4)
            for h in range(2):
                nc.tensor.matmul(
                    bmz[:, ts(h, 512)], lhsT=ones128[:],
                    rhs=sqc[:, ts(h, 512)], start=True, stop=False,
                )
                nc.tensor.matmul(
                    bmz[:, ts(h, 512)], lhsT=negones[:],
                    rhs=prod[:, ts(h, 512)], start=False, stop=True,
                )
            nc.scalar.activation(
                a16[0:1, :], bmz[0:1, 0:SH], Act.Identity, bias=marg[0:1, 0:1]
            )
            # transpose a16 -> acol via PE, single copy out
            tpt = ps.tile([128, 1024], f32, tag="g", bufs=4)
            for m in range(MT):
                nc.tensor.matmul(
                    tpt[:, m : m + 1], lhsT=a16[0:1, ts(m, 128)],
                    rhs=ones1[:], start=True, stop=True,
                )
            nc.scalar.activation(acol[:, 0:MT], tpt[:, 0:MT], Act.Copy)

            # ---- main loop: per-col m-tiles, fold -b into PSUM ----
            for n in range(NG):
                for k, m in enumerate(M_ORDER):
                    is_dve = m in DVE_MS
                    pt = ps.tile([128, 1024], f32, tag="g", bufs=4)
                    # both G halves first (one weight load), then both folds
                    for h in range(2):
                        nc.tensor.matmul(
                            pt[:, ts(h, 512)], lhsT=xT[:, ts(m, 128)],
                            rhs=yTs[n][:, ts(h, 512)],
                            start=True, stop=False,
                        )
                    for h in range(2):
                        nc.tensor.matmul(
                            pt[:, ts(h, 512)], lhsT=negones[:],
                            rhs=sqs[n][:, ts(h, 512)],
                            start=False, stop=True,
                        )
                    if is_dve:
                        di = n * NDVE + DVE_MS.index(m)
                        scr = work.tile([128, 1024], f32, tag="ep_dve")
                        nc.vector.scalar_tensor_tensor(
                            out=scr[:], in0=pt[:], scalar=acol[:, m : m + 1],
                            in1=zeros[:], op0=Alu.add, op1=Alu.max,
                            accum_out=res_d[:, di : di + 1],
                        )
                    else:
                        ai = n * (MT - NDVE) + ACT_MS.index(m)
                        scr = work.tile([128, 1024], f32, tag="ep_act")
                        nc.scalar.activation(
                            scr[:], pt[:], Act.Relu,
                            bias=acol[:, m : m + 1],
                            accum_out=res_a[:, ai : ai + 1],
                        )
                    if k == 6 and n + 2 < NG:
                        do_sq(n + 2)

            nc.sync.dma_start(out_res[:, : NDVE * NG], res_d[:])
            nc.sync.dma_start(out_res[:, NDVE * NG :], res_a[:])

    return nc


def _make_in_maps(x: np.ndarray, y: np.ndarray) -> list:
    import ml_dtypes

    x = np.ascontiguousarray(x, dtype=np.float32)
    y = np.ascontiguousarray(y, dtype=np.float32)
    yb = y.astype(ml_dtypes.bfloat16)
    yT = np.ascontiguousarray(yb.T)
    in_maps = []
    for c in range(NCORES):
        sl = slice(c * SH, (c + 1) * SH)
        xT2 = (2.0 * x[sl]).astype(ml_dtypes.bfloat16).T
        ylT = yb[sl].T
        xyl = np.ascontiguousarray(np.concatenate([xT2, ylT], axis=1))
        in_maps.append({"xyl": xyl, "yT": yT})
    return in_maps


def kernel(x: np.ndarray, y: np.ndarray) -> np.ndarray:
    from concourse.bass_utils import run_bass_kernel_spmd

    x = np.ascontiguousarray(x, dtype=np.float32)
    y = np.ascontiguousarray(y, dtype=np.float32)

    if "nc" not in _cache:
        nc = _build()
        if not nc.is_finalized():
            nc.finalize()
        _cache["nc"] = nc
    nc = _cache["nc"]

    out = run_bass_kernel_spmd(nc, _make_in_maps(x, y), list(range(NCORES)))
    results = out.results

    total = 0.0
    for c in range(NCORES):
        total += np.asarray(results[c]["res"], dtype=np.float64).sum()
    total -= float(N) * float(np.float32(MARGIN))
    return np.float32(total / (float(N) * float(N)))


# revision 22
# speedup vs baseline: 1.4394x; 1.0297x over previous
"""HardTripletLoss (non-hardest branch) on 8 TRN2 NeuronCores.

Math:  loss = mean_{i!=j} relu(d_pos[i] - pdist[i,j] + margin)
  pdist[i,j] = ||x_i||^2 + ||y_j||^2 - 2 x_i.y_j ,  d_pos = diag(pdist)
  =>  per-term: relu(G[i,j] + a[i] - b[j]) with G = 2 x y^T,
      a[i] = margin + b[i] - G[i,i],  b[j] = ||y_j||^2  (xx cancels).
Diagonal (i==j) evaluates to ~relu(margin) = margin; the full unmasked sum is
computed and N*margin subtracted on the host.

Sharding: x rows split across 8 cores, y replicated.  Inputs arrive
pre-transposed/scaled from the host (bf16): xT2 = (2 x_shard)^T [128,1024],
ylT = y_shard^T [128,1024] (bit-identical to the matching yT slice),
yT = y^T [128,8192].  ~2.5 MB HBM per core, all HWDGE.

Per core, per col-pair (4 pairs of 1024-col groups), per m-tile (8):
  PSUM[128,2048] = xT2_m^T @ [yT_n | yT_n1]  chained with  -ones128^T @ sq
  (sq_n = square(yT_n) bf16, prefetched on ACT/DVE alternating)
  so PSUM = G - b over two col groups.  One epilogue op per m-supertile,
  alternating engines (same m => uniform per-partition a):
    DVE: sum_j max(PSUM + a_m, 0)   (STT vs a zeros tile, accum)
    ACT: sum_j relu(PSUM + a_m)     (activation w/ bias, accum)
a-path: sqc = square(ylT); prod = xT2*ylT elementwise; PE computes
bmz = ones@sqc - ones@prod (= b - z2) in one PSUM accumulation;
a16[1,1024] = Identity(bmz[0,:] + margin) bf16 in one ACT op;
PE-transpose (lhsT=a16 chunk, rhs=[1,1] ones) -> acol [128,8] f32.
The fold -ones@sq is the exact negation of +ones@sq (same PE summation
order), keeping a and the epilogue b consistent for the diagonal.
Host: loss = (sum(res) - N*margin) / N^2 in f64.
"""

import sys

if "/opt/trn_rl_repo" not in sys.path:
    sys.path.insert(0, "/opt/trn_rl_repo")

import numpy as np

N, D = 8192, 128
NCORES = 8
SH = N // NCORES          # 1024 x-rows per core
MT = SH // 128            # 8 m-tiles
NG = N // 1024            # 8 col groups of 1024
MARGIN = 0.2
# m-tile -> engine: even m = DVE (max vs 0), odd m = ACT (relu w/ bias).
# ACT tile first in each col (ACT also owns half the squares).
DVE_MS = tuple(m for m in range(MT) if m % 2 == 0)
ACT_MS = [m for m in range(MT) if m not in DVE_MS]
M_ORDER = [1, 0, 3, 2, 5, 4, 7, 6]

_cache = {}


def _build():
    import concourse.mybir as mybir
    from concourse import bacc
    from concourse.tile import TileContext
    from concourse.bass import ts

    f32 = mybir.dt.float32
    bf16 = mybir.dt.bfloat16
    Alu = mybir.AluOpType
    Act = mybir.ActivationFunctionType

    nc = bacc.Bacc()
    xyl_in = nc.declare_dram_parameter("xyl", [128, 2 * SH], bf16, isOutput=False)
    yT_in = nc.declare_dram_parameter("yT", [128, N], bf16, isOutput=False)
    out_res = nc.declare_dram_parameter("res", [128, MT * NG], f32, isOutput=True)
    NDVE = len(DVE_MS)

    def sq_engine(n):
        return "act" if n % 2 == 0 else "dve"

    with TileContext(nc) as tc:
        with (
            tc.tile_pool(name="big", bufs=1) as big,
            tc.tile_pool(name="work", bufs=3) as work,
            tc.tile_pool(name="ps", bufs=1, space="PSUM") as ps,
        ):
            yTs = [big.tile([128, 1024], bf16, name=f"yT{n}") for n in range(NG)]
            xyl = big.tile([128, 2 * SH], bf16)
            sqs = [big.tile([128, 1024], bf16, name=f"sq{n}") for n in range(NG)]
            sqc = big.tile([128, SH], bf16)
            ones128 = big.tile([128, 128], bf16)
            negones = big.tile([128, 128], bf16)
            ones1 = big.tile([1, 1], bf16)
            marg = big.tile([1, 1], f32)
            zeros = big.tile([128, 1024], f32)
            prod = big.tile([128, SH], bf16)
            a16 = big.tile([1, SH], bf16)
            z2r = big.tile([1, SH], f32)
            acol = big.tile([128, MT], f32)
            res_d = big.tile([128, NDVE * NG], f32)
            res_a = big.tile([128, (MT - NDVE) * NG], f32)

            nc.gpsimd.memset(ones128[:], 1.0)
            nc.gpsimd.memset(negones[:], -1.0)
            nc.gpsimd.memset(ones1[:], 1.0)
            nc.gpsimd.memset(marg[:], MARGIN)
            nc.gpsimd.memset(zeros[:], 0.0)

            nc.sync.dma_start(xyl[:], xyl_in[:])
            xT = xyl[:, 0:SH]
            ylT = xyl[:, SH : 2 * SH]
            for n in range(NG):
                nc.sync.dma_start(yTs[n][:], yT_in[:, ts(n, 1024)])

            def do_sq(n):
                if sq_engine(n) == "act":
                    nc.scalar.activation(sqs[n][:], yTs[n][:], Act.Square)
                else:
                    nc.vector.scalar_tensor_tensor(
                        out=sqs[n][:], in0=yTs[n][:],
                        scalar=1.0, in1=yTs[n][:],
                        op0=Alu.mult, op1=Alu.mult,
                    )

            # ---- preamble: a-path first, then first squares ----
            nc.scalar.activation(sqc[:], ylT, Act.Square)
            nc.vector.scalar_tensor_tensor(     # prod on DVE
                out=prod[:], in0=xT, scalar=1.0, in1=ylT,
                op0=Alu.mult, op1=Alu.mult,
            )
            do_sq(0)                     # ACT
            do_sq(1)                     # DVE

            # bmz = b - z2 in one PSUM accumulation
            bmz = ps.tile([128, 1024], f32, tag="g", bufs=4)
            for h in range(2):
                nc.tensor.matmul(
                    bmz[:, ts(h, 512)], lhsT=ones128[:],
                    rhs=sqc[:, ts(h, 512)], start=True, stop=False,
                )
                nc.tensor.matmul(
                    bmz[:, ts(h, 512)], lhsT=negones[:],
                    rhs=prod[:, ts(h, 512)], start=False, stop=True,
                )
            nc.scalar.activation(
                a16[0:1, :], bmz[0:1, 0:SH], Act.Identity, bias=marg[0:1, 0:1]
            )
            def emit_acol_transpose():
                # transpose a16 -> acol via PE, single copy out
                tpt = ps.tile([128, 1024], f32, tag="g", bufs=4)
                for m in range(MT):
                    nc.tensor.matmul(
                        tpt[:, m : m + 1], lhsT=a16[0:1, ts(m, 128)],
                        rhs=ones1[:], start=True, stop=True,
                    )
                nc.scalar.activation(acol[:, 0:MT], tpt[:, 0:MT], Act.Copy)

            # ---- main loop: per-col m-tiles, fold -b into PSUM ----
            def emit_mm(n, m):
                pt = ps.tile([128, 1024], f32, tag="g", bufs=4)
                # both G halves first (one weight load), then both folds
                for h in range(2):
                    nc.tensor.matmul(
                        pt[:, ts(h, 512)], lhsT=xT[:, ts(m, 128)],
                        rhs=yTs[n][:, ts(h, 512)],
                        start=True, stop=False,
                    )
                for h in range(2):
                    nc.tensor.matmul(
                        pt[:, ts(h, 512)], lhsT=negones[:],
                        rhs=sqs[n][:, ts(h, 512)],
                        start=False, stop=True,
                    )
                return pt

            def emit_ep(n, m, pt):
                if m in DVE_MS:
                    di = n * NDVE + DVE_MS.index(m)
                    scr = work.tile([128, 1024], f32, tag="ep_dve")
                    nc.vector.scalar_tensor_tensor(
                        out=scr[:], in0=pt[:], scalar=acol[:, m : m + 1],
                        in1=zeros[:], op0=Alu.add, op1=Alu.max,
                        accum_out=res_d[:, di : di + 1],
                    )
                else:
                    ai = n * (MT - NDVE) + ACT_MS.index(m)
                    scr = work.tile([128, 1024], f32, tag="ep_act")
                    nc.scalar.activation(
                        scr[:], pt[:], Act.Relu,
                        bias=acol[:, m : m + 1],
                        accum_out=res_a[:, ai : ai + 1],
                    )

            # col 0: fill PSUM while the a-chain completes, then epilogues
            pend = [(m, emit_mm(0, m)) for m in M_ORDER[:3]]
            emit_acol_transpose()
            for m, pt in pend:
                emit_ep(0, m, pt)
            for k, m in enumerate(M_ORDER[3:]):
                pt = emit_mm(0, m)
                emit_ep(0, m, pt)
                if k == 3:
                    do_sq(2)

            for n in range(1, NG):
                for k, m in enumerate(M_ORDER):
                    pt = emit_mm(n, m)
                    emit_ep(n, m, pt)
                    if k == 6 and n + 2 < NG:
                        do_sq(n + 2)

            nc.sync.dma_start(out_res[:, : NDVE * NG], res_d[:])
            nc.sync.dma_start(out_res[:, NDVE * NG :], res_a[:])

    return nc


def _make_in_maps(x: np.ndarray, y: np.ndarray) -> list:
    import ml_dtypes

    x = np.ascontiguousarray(x, dtype=np.float32)
    y = np.ascontiguousarray(y, dtype=np.float32)
    yb = y.astype(ml_dtypes.bfloat16)
    yT = np.ascontiguousarray(yb.T)
    in_maps = []
    for c in range(NCORES):
        sl = slice(c * SH, (c + 1) * SH)
        xT2 = (2.0 * x[sl]).astype(ml_dtypes.bfloat16).T
        ylT = yb[sl].T
        xyl = np.ascontiguousarray(np.concatenate([xT2, ylT], axis=1))
        in_maps.append({"xyl": xyl, "yT": yT})
    return in_maps


def kernel(x: np.ndarray, y: np.ndarray) -> np.ndarray:
    from concourse.bass_utils import run_bass_kernel_spmd

    x = np.ascontiguousarray(x, dtype=np.float32)
    y = np.ascontiguousarray(y, dtype=np.float32)

    if "nc" not in _cache:
        nc = _build()
        if not nc.is_finalized():
            nc.finalize()
        _cache["nc"] = nc
    nc = _cache["nc"]

    out = run_bass_kernel_spmd(nc, _make_in_maps(x, y), list(range(NCORES)))
    results = out.results

    total = 0.0
    for c in range(NCORES):
        total += np.asarray(results[c]["res"], dtype=np.float64).sum()
    total -= float(N) * float(np.float32(MARGIN))
    return np.float32(total / (float(N) * float(N)))
